# revision 13
# baseline (speedup 1.0000x reference)
"""Fixed-point saturating quantization (Q4.4, round-half-up) on 8 TRN2 cores.

  out = clip(floor(x/delta + 0.5), -2^7, 2^7 - 1) * delta,  delta = 2^-4

Purely elementwise -> data parallel: the (64, 256, 56, 56) fp32 input is
flattened and split into 8 contiguous, equal chunks (8 batches per core).

Per-core algorithm (bit-exact vs the fp32 reference):
  ACT pass :  y = Copy(16*x + 0.5)          (free affine, fp32, exact)
  DVE pass :  one custom 7-stage DVE op:
      t  = (y + M) - M        M = 1.5*2^23  -> t = RNE(y), exact for |y| < 2^22
      f  = t - (t > y)        -> floor(y)   (comparison yields 1.0/0.0)
      o  = min(max(f * 2^-4, -8.0), 7.9375) -> clip fused with the *delta scale

Raw Bass (no TileContext — its tail-barrier EVSEM/drain instructions don't
compile with this walrus build): a 4-engine ring pipeline with manual
semaphores. sync issues input DMAs (HWDGE), scalar runs the affine, vector
runs the quant op, gpsimd issues output DMAs (SWDGE). NBUF-deep buffers per
stage; the kernel is DMA-bound (~51 MB of HBM traffic per core).
"""

import numpy as np

import concourse.bass as bass
import concourse.mybir as mybir
import concourse.dve_ops as dve_ops
from concourse.bass_utils import run_bass_kernel_spmd
from concourse.dve_spec import C0, C1, C2, Spec, Src0, Zero, lower, maxx, minn
from concourse.dve_uop import DveOpSpec

N_CORES = 8
FULL_SHAPE = (64, 256, 56, 56)
TOTAL = 64 * 256 * 56 * 56          # 51_380_224
PER_CORE = TOTAL // N_CORES         # 6_422_528
P = 128                             # SBUF partitions
F = PER_CORE // P                   # 50_176 elements per partition
T = 6272                            # tile free dim -> 8 tiles of 3.2 MB
NBUF = 4                            # ring depth per stage
TENSORS = "two"                     # x tile + shared y/o tile (DVE in place)

MAGIC = 12582912.0                  # 1.5 * 2^23: RNE-to-integer magic constant
DELTA = 0.0625                      # 2^-4
LO = -8.0                           # -2^7 * delta

_QUANT_OP_NAME = "QUANT_FIXED_ANT"


def _quant_ref(in0, in1, c0, c1, c2):
    """Numpy mirror of the DVE op for CoreSim: one fp32 rounding per stage."""
    a = (in0 + np.float32(c0)).astype(np.float32)
    t = (a - np.float32(c0)).astype(np.float32)
    g = (t > in0).astype(np.float32)
    f = (t - g).astype(np.float32)
    f16 = (f * np.float32(c1)).astype(np.float32)
    ub = np.float32(np.float32(0.0) - np.float32(c2)) - np.float32(c1)
    return np.minimum(np.maximum(f16, np.float32(c2)), ub).astype(np.float32)


def _register_quant_op():
    """Register the custom DVE op in dve_ops.OPS (idempotent per process)."""
    for op in dve_ops.OPS:
        if op.name == _QUANT_OP_NAME:
            return op

    _a = Src0 + C0          # y + M
    _t = _a - C0            # RNE(y)
    _g = _t > Src0          # 1.0 where t > y
    _f = _t - _g            # floor(y)
    _body = minn(maxx(_f * C1, C2), (Zero - C2) - C1)  # clip + *delta (hoisted ub)
    spec = Spec(body=_body, reference=_quant_ref)

    row = dve_ops._CUSTOM_DVE_ROW_BASE + len(dve_ops.OPS)
    shas = {
        ver: DveOpSpec(
            name=_QUANT_OP_NAME, opcode=row, uops=lower(spec, ver=ver), rd1_en=False
        ).sha(ver)
        for ver in ("v3", "v4")
    }
    op = dve_ops.DveOp(_QUANT_OP_NAME, spec, subdim=False, uops_sha=shas)
    dve_ops.OPS.append(op)
    dve_ops.CUSTOM_DVE_SPECS[_QUANT_OP_NAME] = spec
    dve_ops._SUB_OPCODE_FOR_NAME[_QUANT_OP_NAME] = row
    return op


def build_nc(n_tiles=None, tile_f=None, nbuf=None, reps=1, inplace=False,
             tensors=None):
    """reps > 1 re-runs the identical tile loop on the same DRAM data inside
    one NEFF (idempotent) — used only to time the steady-state per-pass cost
    without the per-execute RPC overhead.

    tensors: "three" (x->y->o), "two" (x->y, DVE in place on y), or "one"
    (everything in place on x). inplace=True is shorthand for "one"."""
    if tensors is None:
        tensors = "one" if inplace else TENSORS
    inplace = tensors == "one"
    quant_op = _register_quant_op()
    tile_f = T if tile_f is None else tile_f
    n_tiles = (F // T) if n_tiles is None else n_tiles
    nbuf = NBUF if nbuf is None else nbuf
    n_iter = n_tiles * reps
    dt = mybir.dt.float32

    nc = bass.Bass()
    x_dram = nc.declare_dram_parameter("inp", [P, n_tiles * tile_f], dt, isOutput=False)
    y_dram = nc.declare_dram_parameter("out", [P, n_tiles * tile_f], dt, isOutput=True)

    import contextlib

    with contextlib.ExitStack() as ctx:
        xs = [
            ctx.enter_context(nc.sbuf_tensor(f"xt{j}", [P, tile_f], dt))
            for j in range(nbuf)
        ]
        if tensors == "one":
            ys, os_ = xs, xs
        elif tensors == "two":
            ys = [
                ctx.enter_context(nc.sbuf_tensor(f"yt{j}", [P, tile_f], dt))
                for j in range(nbuf)
            ]
            os_ = ys
        else:
            ys = [
                ctx.enter_context(nc.sbuf_tensor(f"yt{j}", [P, tile_f], dt))
                for j in range(nbuf)
            ]
            os_ = [
                ctx.enter_context(nc.sbuf_tensor(f"ot{j}", [P, tile_f], dt))
                for j in range(nbuf)
            ]
        # Per-slot DMA semaphores: HWDGE/SWDGE fan DMAs out across queues, so
        # completions of different dma_starts are unordered — a cumulative
        # counter would race. Within one ring slot DMAs are serialized by the
        # ring dependencies, so a per-slot counter is race-free.
        sem_in = [
            ctx.enter_context(nc.semaphore(f"sem_in{j}")) for j in range(nbuf)
        ]
        sem_out = [
            ctx.enter_context(nc.semaphore(f"sem_out{j}")) for j in range(nbuf)
        ]
        sem_act = ctx.enter_context(nc.semaphore("sem_act"))
        sem_dve = ctx.enter_context(nc.semaphore("sem_dve"))
        block_cm = nc.Block()
        block = block_cm.__enter__()

        @block.sync
        def _(sync):
            for i in range(n_iter):
                k, m = i % nbuf, i // nbuf
                if i >= nbuf:
                    if tensors == "one":
                        # x[k] reusable once tile i-nbuf's store DMA finished
                        sync.wait_ge(sem_out[k], m * 16)
                    else:
                        # x[k] reusable once ACT consumed tile i-nbuf
                        sync.wait_ge(sem_act, i - nbuf + 1)
                sync.dma_start(
                    out=xs[k][:], in_=x_dram[:, bass.ts(i % n_tiles, tile_f)]
                ).then_inc(sem_in[k], 16)

        @block.scalar
        def _(scalar):
            for i in range(n_iter):
                k, m = i % nbuf, i // nbuf
                scalar.wait_ge(sem_in[k], (m + 1) * 16)
                if i >= nbuf:
                    if tensors == "two":
                        # y[k] (== o[k]) writable once its store DMA finished
                        scalar.wait_ge(sem_out[k], m * 16)
                    elif tensors == "three":
                        # y[k] may be overwritten once DVE consumed tile i-nbuf
                        scalar.wait_ge(sem_dve, i - nbuf + 1)
                scalar.activation(
                    ys[k][:], xs[k][:], mybir.ActivationFunctionType.Copy,
                    bias=0.5, scale=16.0,
                ).then_inc(sem_act, 1)

        @block.vector
        def _(vector):
            for i in range(n_iter):
                k, m = i % nbuf, i // nbuf
                vector.wait_ge(sem_act, i + 1)
                if i >= nbuf and tensors == "three":
                    # o[k] may be overwritten once its store DMA finished
                    vector.wait_ge(sem_out[k], m * 16)
                vector._custom_dve(
                    quant_op, out=os_[k][:], in0=ys[k][:],
                    s0=MAGIC, s1=DELTA, imm2=LO,
                ).then_inc(sem_dve, 1)

        @block.gpsimd
        def _(gpsimd):
            for i in range(n_iter):
                k = i % nbuf
                gpsimd.wait_ge(sem_dve, i + 1)
                gpsimd.dma_start(
                    out=y_dram[:, bass.ts(i % n_tiles, tile_f)], in_=os_[k][:]
                ).then_inc(sem_out[k], 16)
            # Once every store has landed, everything upstream is complete.
            for k in range(nbuf):
                rounds = (n_iter - k + nbuf - 1) // nbuf
                gpsimd.wait_ge(sem_out[k], rounds * 16)

        block_cm.__exit__(None, None, None)
        # Tail reset (emitted in `main`, after each engine returns from the
        # block): zero all semaphores + DMA state so the NEFF can be executed
        # again (raw bass has no Tile tail barrier; stale sem values would
        # deadlock/corrupt a re-execute). Barriers are self-resetting.
        nc.all_engine_barrier()
        all_sems = [s.num for s in (*sem_in, *sem_out, sem_act, sem_dve)]
        lo, hi = min(all_sems), max(all_sems)
        assert hi - lo + 1 == len(all_sems), "semaphores not contiguous"
        nc.gpsimd.dma_reset(range(lo, hi + 1))
        nc.gpsimd.sem_clear(range(lo, hi + 1))
        nc.all_engine_barrier()

    # Fill in the raw ISA bytes of InstCustomDveAnt — walrus rejects the
    # instruction ("ISA wrong length") if this lowering pass hasn't run.
    mybir.codegen_inst_isa_subclasses(nc)
    return nc


def _run(inp: np.ndarray, **spmd_kwargs):
    flat = np.ascontiguousarray(inp, dtype=np.float32).reshape(-1)
    in_maps = [
        {"inp": flat[c * PER_CORE : (c + 1) * PER_CORE].reshape(P, F)}
        for c in range(N_CORES)
    ]
    nc = build_nc()
    return run_bass_kernel_spmd(nc, in_maps, list(range(N_CORES)), **spmd_kwargs)


def kernel(inp: np.ndarray) -> np.ndarray:
    res = _run(inp).results
    out = np.concatenate(
        [np.asarray(res[c]["out"], dtype=np.float32).reshape(-1) for c in range(N_CORES)]
    )
    return out.reshape(FULL_SHAPE)


if __name__ == "__main__":
    rng = np.random.default_rng(0)
    x = rng.standard_normal(FULL_SHAPE, dtype=np.float32)
    out = kernel(x)
    delta = np.float32(2.0 ** -4)
    q = np.floor(x / delta + np.float32(0.5))
    q = np.clip(q, np.float32(-128.0), np.float32(127.0))
    ref = (q * delta).astype(np.float32)
    print("mismatches:", int((out != ref).sum()), "/", ref.size)


# revision 15
# speedup vs baseline: 1.0756x; 1.0756x over previous
"""Fixed-point saturating quantization (Q4.4, round-half-up) on 8 TRN2 cores.

  out = clip(floor(x/delta + 0.5), -2^7, 2^7 - 1) * delta,  delta = 2^-4

Purely elementwise -> data parallel: the (64, 256, 56, 56) fp32 input is
flattened and split into 8 contiguous, equal chunks (8 batches per core).

Per-core algorithm (bit-exact vs the fp32 reference):
  ACT pass :  y = Copy(16*x + 0.5)          (free affine, fp32, exact)
  DVE pass :  one custom 7-stage DVE op:
      t  = (y + M) - M        M = 1.5*2^23  -> t = RNE(y), exact for |y| < 2^22
      f  = t - (t > y)        -> floor(y)   (comparison yields 1.0/0.0)
      o  = min(max(f * 2^-4, -8.0), 7.9375) -> clip fused with the *delta scale

Raw Bass (no TileContext — its tail-barrier EVSEM/drain instructions don't
compile with this walrus build): a 4-engine ring pipeline with manual
semaphores. sync issues input DMAs (HWDGE), scalar runs the affine, vector
runs the quant op, gpsimd issues output DMAs (SWDGE). NBUF-deep buffers per
stage; the kernel is DMA-bound (~51 MB of HBM traffic per core).
"""

import numpy as np

import concourse.bass as bass
import concourse.mybir as mybir
import concourse.dve_ops as dve_ops
from concourse.bass_utils import run_bass_kernel_spmd
from concourse.dve_spec import C0, C1, C2, Spec, Src0, Zero, lower, maxx, minn
from concourse.dve_uop import DveOpSpec

N_CORES = 8
FULL_SHAPE = (64, 256, 56, 56)
TOTAL = 64 * 256 * 56 * 56          # 51_380_224
PER_CORE = TOTAL // N_CORES         # 6_422_528
P = 128                             # SBUF partitions
F = PER_CORE // P                   # 50_176 elements per partition
T = 3136                            # tile free dim -> 16 tiles of 1.6 MB
NBUF = 6                            # ring depth per stage
TENSORS = "two"                     # x tile + shared y/o tile (DVE in place)

MAGIC = 12582912.0                  # 1.5 * 2^23: RNE-to-integer magic constant
DELTA = 0.0625                      # 2^-4
LO = -8.0                           # -2^7 * delta

_QUANT_OP_NAME = "QUANT_FIXED_ANT"


def _quant_ref(in0, in1, c0, c1, c2):
    """Numpy mirror of the DVE op for CoreSim: one fp32 rounding per stage."""
    a = (in0 + np.float32(c0)).astype(np.float32)
    t = (a - np.float32(c0)).astype(np.float32)
    g = (t > in0).astype(np.float32)
    f = (t - g).astype(np.float32)
    f16 = (f * np.float32(c1)).astype(np.float32)
    ub = np.float32(np.float32(0.0) - np.float32(c2)) - np.float32(c1)
    return np.minimum(np.maximum(f16, np.float32(c2)), ub).astype(np.float32)


def _register_quant_op():
    """Register the custom DVE op in dve_ops.OPS (idempotent per process)."""
    for op in dve_ops.OPS:
        if op.name == _QUANT_OP_NAME:
            return op

    _a = Src0 + C0          # y + M
    _t = _a - C0            # RNE(y)
    _g = _t > Src0          # 1.0 where t > y
    _f = _t - _g            # floor(y)
    _body = minn(maxx(_f * C1, C2), (Zero - C2) - C1)  # clip + *delta (hoisted ub)
    spec = Spec(body=_body, reference=_quant_ref)

    row = dve_ops._CUSTOM_DVE_ROW_BASE + len(dve_ops.OPS)
    shas = {
        ver: DveOpSpec(
            name=_QUANT_OP_NAME, opcode=row, uops=lower(spec, ver=ver), rd1_en=False
        ).sha(ver)
        for ver in ("v3", "v4")
    }
    op = dve_ops.DveOp(_QUANT_OP_NAME, spec, subdim=False, uops_sha=shas)
    dve_ops.OPS.append(op)
    dve_ops.CUSTOM_DVE_SPECS[_QUANT_OP_NAME] = spec
    dve_ops._SUB_OPCODE_FOR_NAME[_QUANT_OP_NAME] = row
    return op


def build_nc(n_tiles=None, tile_f=None, nbuf=None, reps=1, inplace=False,
             tensors=None):
    """reps > 1 re-runs the identical tile loop on the same DRAM data inside
    one NEFF (idempotent) — used only to time the steady-state per-pass cost
    without the per-execute RPC overhead.

    tensors: "three" (x->y->o), "two" (x->y, DVE in place on y), or "one"
    (everything in place on x). inplace=True is shorthand for "one"."""
    if tensors is None:
        tensors = "one" if inplace else TENSORS
    inplace = tensors == "one"
    quant_op = _register_quant_op()
    tile_f = T if tile_f is None else tile_f
    n_tiles = (F // tile_f) if n_tiles is None else n_tiles
    nbuf = NBUF if nbuf is None else nbuf
    n_iter = n_tiles * reps
    dt = mybir.dt.float32

    nc = bass.Bass()
    x_dram = nc.declare_dram_parameter("inp", [P, n_tiles * tile_f], dt, isOutput=False)
    y_dram = nc.declare_dram_parameter("out", [P, n_tiles * tile_f], dt, isOutput=True)

    import contextlib

    with contextlib.ExitStack() as ctx:
        xs = [
            ctx.enter_context(nc.sbuf_tensor(f"xt{j}", [P, tile_f], dt))
            for j in range(nbuf)
        ]
        if tensors == "one":
            ys, os_ = xs, xs
        elif tensors == "two":
            ys = [
                ctx.enter_context(nc.sbuf_tensor(f"yt{j}", [P, tile_f], dt))
                for j in range(nbuf)
            ]
            os_ = ys
        else:
            ys = [
                ctx.enter_context(nc.sbuf_tensor(f"yt{j}", [P, tile_f], dt))
                for j in range(nbuf)
            ]
            os_ = [
                ctx.enter_context(nc.sbuf_tensor(f"ot{j}", [P, tile_f], dt))
                for j in range(nbuf)
            ]
        # Per-slot DMA semaphores: HWDGE/SWDGE fan DMAs out across queues, so
        # completions of different dma_starts are unordered — a cumulative
        # counter would race. Within one ring slot DMAs are serialized by the
        # ring dependencies, so a per-slot counter is race-free.
        sem_in = [
            ctx.enter_context(nc.semaphore(f"sem_in{j}")) for j in range(nbuf)
        ]
        sem_out = [
            ctx.enter_context(nc.semaphore(f"sem_out{j}")) for j in range(nbuf)
        ]
        sem_act = ctx.enter_context(nc.semaphore("sem_act"))
        sem_dve = ctx.enter_context(nc.semaphore("sem_dve"))
        block_cm = nc.Block()
        block = block_cm.__enter__()

        @block.sync
        def _(sync):
            for i in range(n_iter):
                k, m = i % nbuf, i // nbuf
                if i >= nbuf:
                    if tensors == "one":
                        # x[k] reusable once tile i-nbuf's store DMA finished
                        sync.wait_ge(sem_out[k], m * 16)
                    else:
                        # x[k] reusable once ACT consumed tile i-nbuf
                        sync.wait_ge(sem_act, i - nbuf + 1)
                sync.dma_start(
                    out=xs[k][:], in_=x_dram[:, bass.ts(i % n_tiles, tile_f)]
                ).then_inc(sem_in[k], 16)

        @block.scalar
        def _(scalar):
            for i in range(n_iter):
                k, m = i % nbuf, i // nbuf
                scalar.wait_ge(sem_in[k], (m + 1) * 16)
                if i >= nbuf:
                    if tensors == "two":
                        # y[k] (== o[k]) writable once its store DMA finished
                        scalar.wait_ge(sem_out[k], m * 16)
                    elif tensors == "three":
                        # y[k] may be overwritten once DVE consumed tile i-nbuf
                        scalar.wait_ge(sem_dve, i - nbuf + 1)
                scalar.activation(
                    ys[k][:], xs[k][:], mybir.ActivationFunctionType.Copy,
                    bias=0.5, scale=16.0,
                ).then_inc(sem_act, 1)

        @block.vector
        def _(vector):
            for i in range(n_iter):
                k, m = i % nbuf, i // nbuf
                vector.wait_ge(sem_act, i + 1)
                if i >= nbuf and tensors == "three":
                    # o[k] may be overwritten once its store DMA finished
                    vector.wait_ge(sem_out[k], m * 16)
                vector._custom_dve(
                    quant_op, out=os_[k][:], in0=ys[k][:],
                    s0=MAGIC, s1=DELTA, imm2=LO,
                ).then_inc(sem_dve, 1)

        @block.gpsimd
        def _(gpsimd):
            for i in range(n_iter):
                k = i % nbuf
                gpsimd.wait_ge(sem_dve, i + 1)
                gpsimd.dma_start(
                    out=y_dram[:, bass.ts(i % n_tiles, tile_f)], in_=os_[k][:]
                ).then_inc(sem_out[k], 16)
            # Once every store has landed, everything upstream is complete.
            for k in range(nbuf):
                rounds = (n_iter - k + nbuf - 1) // nbuf
                gpsimd.wait_ge(sem_out[k], rounds * 16)

        block_cm.__exit__(None, None, None)
        # Tail reset (emitted in `main`, after each engine returns from the
        # block): zero all semaphores + DMA state so the NEFF can be executed
        # again (raw bass has no Tile tail barrier; stale sem values would
        # deadlock/corrupt a re-execute). Barriers are self-resetting.
        nc.all_engine_barrier()
        all_sems = [s.num for s in (*sem_in, *sem_out, sem_act, sem_dve)]
        lo, hi = min(all_sems), max(all_sems)
        assert hi - lo + 1 == len(all_sems), "semaphores not contiguous"
        nc.gpsimd.dma_reset(range(lo, hi + 1))
        nc.gpsimd.sem_clear(range(lo, hi + 1))
        nc.all_engine_barrier()

    # Fill in the raw ISA bytes of InstCustomDveAnt — walrus rejects the
    # instruction ("ISA wrong length") if this lowering pass hasn't run.
    mybir.codegen_inst_isa_subclasses(nc)
    return nc


def _run(inp: np.ndarray, **spmd_kwargs):
    flat = np.ascontiguousarray(inp, dtype=np.float32).reshape(-1)
    in_maps = [
        {"inp": flat[c * PER_CORE : (c + 1) * PER_CORE].reshape(P, F)}
        for c in range(N_CORES)
    ]
    nc = build_nc()
    return run_bass_kernel_spmd(nc, in_maps, list(range(N_CORES)), **spmd_kwargs)


def kernel(inp: np.ndarray) -> np.ndarray:
    res = _run(inp).results
    out = np.concatenate(
        [np.asarray(res[c]["out"], dtype=np.float32).reshape(-1) for c in range(N_CORES)]
    )
    return out.reshape(FULL_SHAPE)


if __name__ == "__main__":
    rng = np.random.default_rng(0)
    x = rng.standard_normal(FULL_SHAPE, dtype=np.float32)
    out = kernel(x)
    delta = np.float32(2.0 ** -4)
    q = np.floor(x / delta + np.float32(0.5))
    q = np.clip(q, np.float32(-128.0), np.float32(127.0))
    ref = (q * delta).astype(np.float32)
    print("mismatches:", int((out != ref).sum()), "/", ref.size)
